# revision 2
# baseline (speedup 1.0000x reference)
"""Trainium2 Bass kernel for CrossGeometricStructureEmbedding.

Math per point n, anchor k:
  d_idx = |p_n - a_k| / 0.2, a_idx = atan2(|u x v|, u.v) * 180/(15*pi)
  out[n] = max_k(Wd@emb(d_idx)) + max_k(Wa@emb(a_idx)) + bd + ba

Approach (8 cores, 512 points each):
  emb(x)@W.T is compressed through a Fourier-extension basis
  emb(x) ~= B(x) @ C with B_j(x) = sin(2*pi*(f_j*x + phi_j)) at machine
  precision (64 d-rows / 32 a-rows), so each pair needs only 96 sins
  and the projection contraction drops 256 -> 96. No arccos/Chebyshev
  chain is needed: the basis is evaluated directly in dist / angle.

  Per 512-pair chunk (8 points x 64 anchors), engine split is chosen
  from walrus-legal ops only (Pool cannot touch PSUM or do TT-max):
    SP    x broadcast to 96 partitions as one wide stride-0 DMA/proj
    Pool  t = f_j*x + phi_j (per-partition scalars); frac = t - it
    ACT   it = int32(round(t)); basis = Sin(2*pi*frac)
    PE    4 f32r projection matmuls (96 -> 2x256 dims)
    DVE   k-max of both psums (TensorReduce, the only legal max)
  The x rows are relaid out point-major -> pair-major once through
  DRAM (2 write DMAs); per-chunk broadcasts read DRAM directly.
"""
import sys

sys.path.insert(0, "/opt/trn_rl_repo")

import numpy as np
import concourse.bacc as bacc
import concourse.bass as bass
import concourse.tile as tile
from concourse import mybir
from concourse.bass_utils import run_bass_kernel_spmd

F32 = mybir.dt.float32
F32R = mybir.dt.float32r
I32 = mybir.dt.int32
AF = mybir.ActivationFunctionType
OP = mybir.AluOpType

NCORES = 8
N = 4096
NC_PTS = N // NCORES          # 512 points per core
K = 64
HIDDEN = 256
SIGMA_D = 0.2
SIGMA_A = 15.0
FACTOR_A = 180.0 / (SIGMA_A * np.pi)
TWO_PI = float(2.0 * np.pi)

# Fourier-extension basis: emb(x) ~= sin(2pi(f_j x + phi_j)) @ C
J_D, PEXT_D, LO_D, HI_D = 31, 90.0, -0.5, 44.5
J_A, PEXT_A, LO_A, HI_A = 15, 26.0, -0.5, 12.5
M_D, M_A = 2 * J_D + 2, 2 * J_A + 2          # 64, 32
MB = M_D + M_A

_DIV = np.exp(np.arange(0, HIDDEN, 2) * (-np.log(10000.0) / HIDDEN))


def _fourier_rows(J, pext):
    freqs = [0.0]
    phases = [0.25]
    for j in range(1, J + 1):
        freqs += [j / pext, j / pext]
        phases += [0.0, 0.25]
    freqs.append((J + 1) / pext)
    phases.append(0.0)
    return np.array(freqs), np.array(phases)


def _fit(lo, hi, J, pext, grid_n=8000):
    f, p = _fourier_rows(J, pext)
    xg = np.linspace(lo, hi, grid_n)
    B = np.sin(2 * np.pi * (xg[:, None] * f[None, :] + p[None, :]))
    om = xg[:, None] * _DIV
    E = np.stack([np.sin(om), np.cos(om)], -1).reshape(grid_n, HIDDEN)
    C, *_ = np.linalg.lstsq(B, E, rcond=None)
    return C, f, p


_C_D, _F_D, _P_D = _fit(LO_D, HI_D, J_D, PEXT_D)
_C_A, _F_A, _P_A = _fit(LO_A, HI_A, J_A, PEXT_A)

_NC_CACHE = {}


def _build_nc():
    nc = bacc.Bacc("TRN2", target_bir_lowering=False, debug=False,
                   num_devices=NCORES)
    pts = nc.declare_dram_parameter("pts", [128, 12], F32, isOutput=False)
    nab = nc.declare_dram_parameter("nab", [128, 6, K], F32, isOutput=False)
    wlhs = nc.declare_dram_parameter("wlhs", [MB, 512], F32R, isOutput=False)
    sjp = nc.declare_dram_parameter("sjp", [MB, 2], F32, isOutput=False)
    biasd = nc.declare_dram_parameter("biasd", [128, 2], F32, isOutput=False)
    outT = nc.declare_dram_parameter("outT", [2, 128, 512], F32, isOutput=True)

    NCH = 64

    with tile.TileContext(nc) as tc:
        with (
            tc.tile_pool(name="singles", bufs=1) as sg,
            tc.tile_pool(name="geom", bufs=1) as gm,
            tc.tile_pool(name="dram", bufs=1, space="DRAM") as dr,
            tc.tile_pool(name="pj", bufs=2, space="PSUM") as pj,
            tc.tile_pool(name="tbp", bufs=3) as tbp,
            tc.tile_pool(name="ttp", bufs=3) as ttp,
            tc.tile_pool(name="itp", bufs=3) as itp,
            tc.tile_pool(name="rtp", bufs=3) as rtp,
            tc.tile_pool(name="btp", bufs=3) as btp,
        ):
            pts_sb = sg.tile([128, 12], F32, name="pts_sb")
            nab_sb = sg.tile([128, 6, K], F32, name="nab_sb")
            wlhs_sb = sg.tile([MB, 512], F32R, name="wlhs_sb")
            sjp_sb = sg.tile([MB, 2], F32, name="sjp_sb")
            bias_sb = sg.tile([128, 2], F32, name="bias_sb")
            mx_all = sg.tile([128, 4, 512], F32, name="mx_all")
            thd = dr.tile([2, 8, 4096], F32, name="thd")

            nc.sync.dma_start(pts_sb[:], pts[:])
            nc.sync.dma_start(nab_sb[:], nab[:])
            nc.scalar.dma_start(wlhs_sb[:], wlhs[:])
            nc.sync.dma_start(sjp_sb[:], sjp[:])
            nc.sync.dma_start(bias_sb[:], biasd[:])

            # ---------- geometry ([128, 256] wide) -------------------------
            # dedicated scratch per product term: no false deps, the cross/
            # dot chain and the dist chain run fully in parallel
            W = 4 * K  # 256
            u6 = gm.tile([128, 6, W], F32, name="u6")
            engs = [nc.vector, nc.gpsimd]

            m0 = gm.tile([128, 8, W], F32, name="m0")
            ta = gm.tile([128, W], F32, name="ta")
            tb = gm.tile([128, W], F32, name="tb")
            cx = gm.tile([128, W], F32, name="cx")
            cy = gm.tile([128, W], F32, name="cy")
            cz = gm.tile([128, W], F32, name="cz")
            dt_ = gm.tile([128, W], F32, name="dt_")
            xpair = gm.tile([128, 2, W], F32, name="xpair")

            # warm the sqrt act table while the loads are in flight
            nc.scalar.square(ta[0:1, 0:4], pts_sb[0:1, 0:4])
            nc.scalar.activation(tb[0:1, 0:4], ta[0:1, 0:4], AF.Sqrt)

            for c in range(6):
                for g in range(4):
                    engs[(c * 4 + g) % 2].tensor_scalar_add(
                        u6[:, c, g * K:(g + 1) * K],
                        nab_sb[:, c, :],
                        pts_sb[:, g * 3 + (c % 3):g * 3 + (c % 3) + 1],
                    )

            # cross products (Pool mults, DVE subs; all scratch distinct)
            nc.gpsimd.tensor_tensor(out=m0[:, 0, :], in0=u6[:, 1, :], in1=u6[:, 5, :], op=OP.mult)
            nc.gpsimd.tensor_tensor(out=m0[:, 1, :], in0=u6[:, 2, :], in1=u6[:, 4, :], op=OP.mult)
            nc.gpsimd.tensor_tensor(out=m0[:, 2, :], in0=u6[:, 2, :], in1=u6[:, 3, :], op=OP.mult)
            nc.gpsimd.tensor_tensor(out=m0[:, 3, :], in0=u6[:, 0, :], in1=u6[:, 5, :], op=OP.mult)
            nc.gpsimd.tensor_tensor(out=m0[:, 4, :], in0=u6[:, 0, :], in1=u6[:, 4, :], op=OP.mult)
            nc.gpsimd.tensor_tensor(out=m0[:, 5, :], in0=u6[:, 1, :], in1=u6[:, 3, :], op=OP.mult)
            nc.vector.tensor_tensor(out=cx[:], in0=m0[:, 0, :], in1=m0[:, 1, :], op=OP.subtract)
            nc.vector.tensor_tensor(out=cy[:], in0=m0[:, 2, :], in1=m0[:, 3, :], op=OP.subtract)
            nc.vector.tensor_tensor(out=cz[:], in0=m0[:, 4, :], in1=m0[:, 5, :], op=OP.subtract)

            # dist chain (ACT squares + DVE adds), parallel to the cross chain
            nc.scalar.square(ta[:], u6[:, 0, :])
            nc.scalar.square(tb[:], u6[:, 1, :])
            nc.vector.tensor_tensor(out=ta[:], in0=ta[:], in1=tb[:], op=OP.add)
            nc.scalar.square(tb[:], u6[:, 2, :])
            nc.vector.tensor_tensor(out=ta[:], in0=ta[:], in1=tb[:], op=OP.add)
            nc.scalar.activation(xpair[:, 0, :], ta[:], AF.Sqrt)   # dist

            # thd[proj, rr=2g+h, (p%64)*64 + k] = xpair[p, proj, g*64+k]
            def wr(proj, h, eng):
                src = xpair[64 * h:64 * h + 64, proj, :].rearrange(
                    "p (g k) -> p g k", k=K)
                dst = thd[proj].rearrange(
                    "(g h) (p k) -> h p g k", h=2, k=K)[h]
                eng.dma_start(out=dst, in_=src)

            wr(0, 0, nc.sync)
            wr(0, 1, nc.scalar)

            # dot products (Pool) and |cross|^2 (ACT squares)
            nc.gpsimd.tensor_tensor(out=m0[:, 6, :], in0=u6[:, 0, :], in1=u6[:, 3, :], op=OP.mult)
            nc.gpsimd.tensor_tensor(out=m0[:, 7, :], in0=u6[:, 1, :], in1=u6[:, 4, :], op=OP.mult)
            nc.gpsimd.tensor_tensor(out=ta[:], in0=u6[:, 2, :], in1=u6[:, 5, :], op=OP.mult)
            nc.gpsimd.tensor_tensor(out=dt_[:], in0=m0[:, 6, :], in1=m0[:, 7, :], op=OP.add)
            nc.gpsimd.tensor_tensor(out=dt_[:], in0=dt_[:], in1=ta[:], op=OP.add)
            nc.gpsimd.tensor_tensor(out=cx[:], in0=cx[:], in1=cx[:], op=OP.mult)
            nc.gpsimd.tensor_tensor(out=cy[:], in0=cy[:], in1=cy[:], op=OP.mult)
            nc.gpsimd.tensor_tensor(out=cx[:], in0=cx[:], in1=cy[:], op=OP.add)
            nc.gpsimd.tensor_tensor(out=cz[:], in0=cz[:], in1=cz[:], op=OP.mult)
            nc.gpsimd.tensor_tensor(out=cx[:], in0=cx[:], in1=cz[:], op=OP.add)

            nc.scalar.activation(cy[:], cx[:], AF.Sqrt)            # |cross|
            # angle = arctan(|cross|/dot) + pi*(dot<0)
            nc.vector.reciprocal(cz[:], dt_[:])
            nc.vector.tensor_tensor(out=cy[:], in0=cy[:], in1=cz[:], op=OP.mult)
            nc.scalar.activation(cy[:], cy[:], AF.Arctan)
            nc.vector.tensor_scalar(out=cz[:], in0=dt_[:], scalar1=0.0,
                                    scalar2=None, op0=OP.is_lt)
            nc.vector.scalar_tensor_tensor(out=xpair[:, 1, :], in0=cz[:],
                                           scalar=float(np.pi), in1=cy[:],
                                           op0=OP.mult, op1=OP.add)
            wr(1, 0, nc.sync)
            wr(1, 1, nc.scalar)

            # ---------- pipelined chunk loop: 64 chunks of 512 pairs --------
            tb6 = {}
            tt = {}
            it = {}
            rt = {}
            bt = {}
            psA = {}
            psB = {}

            def s_bc(c):
                # broadcast this chunk's x rows to 96 partitions (stride-0)
                rr, cc = divmod(c, 8)
                t_ = tbp.tile([MB, 512], F32, name="tb6")
                for proj, (p0, cnt) in enumerate(((0, M_D), (M_D, M_A))):
                    srow = thd[proj, rr, :]
                    ap = bass.AP(tensor=srow.tensor,
                                 offset=srow.offset + cc * 512,
                                 ap=[[0, cnt], [1, 512]])
                    nc.sync.dma_start(out=t_[p0:p0 + cnt, :], in_=ap)
                tb6[c] = t_

            def s_aff(c):
                # t = f_j * x + phi_j  (per-partition scalars, on Pool)
                tt_ = ttp.tile([MB, 512], F32, name="tt")
                nc.gpsimd.tensor_scalar(out=tt_[:], in0=tb6[c][:],
                                        scalar1=sjp_sb[:, 0:1],
                                        scalar2=sjp_sb[:, 1:2],
                                        op0=OP.mult, op1=OP.add)
                tt[c] = tt_
                tb6.pop(c)

            def s_cast(c):
                it_ = itp.tile([MB, 512], I32, name="it")
                nc.scalar.copy(it_[:], tt[c][:])
                it[c] = it_

            def s_sub(c):
                rt_ = rtp.tile([MB, 512], F32, name="rt")
                nc.gpsimd.tensor_tensor(out=rt_[:], in0=tt[c][:],
                                        in1=it[c][:], op=OP.subtract)
                rt[c] = rt_
                tt.pop(c)
                it.pop(c)

            def s_sin(c):
                bt_ = btp.tile([MB, 512], F32R, name="bt")
                nc.scalar.activation(bt_[:], rt[c][:], AF.Sin, scale=TWO_PI)
                bt[c] = bt_
                rt.pop(c)

            def s_proj(c):
                # all 4 projection matmuls into one 4-bank psum tile so the
                # k-max is a single TensorReduce
                ps_ = pj.tile([128, 2048], F32, name="ps")
                b = bt[c]
                nc.tensor.matmul(ps_[:, 0:512], wlhs_sb[0:M_D, 0:128],
                                 b[0:M_D, :], start=True, stop=True)
                nc.tensor.matmul(ps_[:, 512:1024], wlhs_sb[0:M_D, 128:256],
                                 b[0:M_D, :], start=True, stop=True)
                nc.tensor.matmul(ps_[:, 1024:1536], wlhs_sb[M_D:MB, 256:384],
                                 b[M_D:MB, :], start=True, stop=True)
                nc.tensor.matmul(ps_[:, 1536:2048], wlhs_sb[M_D:MB, 384:512],
                                 b[M_D:MB, :], start=True, stop=True)
                psA[c] = ps_

            def s_red(c):
                v = psA[c].rearrange("p (t n k) -> p t n k", t=4, k=K)
                nc.vector.tensor_reduce(
                    mx_all[:, :, c * 8:(c + 1) * 8], v[:],
                    axis=mybir.AxisListType.X, op=OP.max)
                psA.pop(c)
                bt.pop(c)

            o0 = gm.tile([128, 512], F32, name="o0")
            o1 = gm.tile([128, 512], F32, name="o1")

            def s_fin(g8):
                cs = slice(g8 * 32, (g8 + 1) * 32)
                nc.gpsimd.tensor_tensor(out=o0[:, cs], in0=mx_all[:, 0, cs],
                                        in1=mx_all[:, 2, cs], op=OP.add)
                nc.gpsimd.tensor_scalar_add(o0[:, cs], o0[:, cs],
                                            bias_sb[:, 0:1])
                nc.gpsimd.tensor_tensor(out=o1[:, cs], in0=mx_all[:, 1, cs],
                                        in1=mx_all[:, 3, cs], op=OP.add)
                nc.gpsimd.tensor_scalar_add(o1[:, cs], o1[:, cs],
                                            bias_sb[:, 1:2])
                nc.scalar.dma_start(out=outT[0, :, cs], in_=o0[:, cs])
                nc.scalar.dma_start(out=outT[1, :, cs], in_=o1[:, cs])

            for s in range(NCH + 5):
                if 5 <= s:
                    s_red(s - 5)
                    if (s - 5) % 4 == 3:
                        s_fin((s - 5) // 4)
                if 1 <= s <= NCH:
                    s_aff(s - 1)
                if 2 <= s <= NCH + 1:
                    s_cast(s - 2)
                    s_sub(s - 2)
                if 3 <= s <= NCH + 2:
                    s_sin(s - 3)
                if s < NCH:
                    s_bc(s)
                if 4 <= s <= NCH + 3:
                    s_proj(s - 4)

    nc.compile()
    return nc


def _host_inputs(points, anchor_points, cor_score, Wa, ba, Wd, bd):
    p = np.ascontiguousarray(points[0], dtype=np.float32)         # (4096, 3)
    a = np.ascontiguousarray(anchor_points[0], dtype=np.float32)  # (64, 3)

    nab = np.empty((128, 6, K), np.float32)
    nab[:, 0:3, :] = -a.T[None, :, :]
    nab[:, 3:6, :] = -np.roll(a, -1, axis=0).T[None, :, :]

    G_d = (_C_D @ np.asarray(Wd, np.float64).T).astype(np.float32)  # (64, 256)
    G_a = (_C_A @ np.asarray(Wa, np.float64).T).astype(np.float32)  # (32, 256)
    wlhs = np.zeros((MB, 512), np.float32)
    wlhs[0:M_D, 0:128] = G_d[:, 0:128]
    wlhs[0:M_D, 128:256] = G_d[:, 128:256]
    wlhs[M_D:MB, 256:384] = G_a[:, 0:128]
    wlhs[M_D:MB, 384:512] = G_a[:, 128:256]

    sjp = np.zeros((MB, 2), np.float32)
    sjp[0:M_D, 0] = _F_D / SIGMA_D       # basis in dist units
    sjp[M_D:MB, 0] = _F_A * FACTOR_A     # basis in raw-angle units
    sjp[0:M_D, 1] = _P_D
    sjp[M_D:MB, 1] = _P_A

    bsum = (np.asarray(bd) + np.asarray(ba)).astype(np.float32)
    biasd = np.stack([bsum[0:128], bsum[128:256]], axis=1).copy()  # (128, 2)

    in_maps = []
    for core in range(NCORES):
        pc = p[core * NC_PTS:(core + 1) * NC_PTS]   # (512, 3)
        ptsv = pc.reshape(4, 128, 3).transpose(1, 0, 2).reshape(128, 12)
        in_maps.append({
            "pts": np.ascontiguousarray(ptsv),
            "nab": nab,
            "wlhs": wlhs,
            "sjp": sjp,
            "biasd": biasd,
        })
    return in_maps


def kernel(points, anchor_points, cor_score, Wa, ba, Wd, bd, _timing=None):
    if "nc" not in _NC_CACHE:
        _NC_CACHE["nc"] = _build_nc()
    nc = _NC_CACHE["nc"]
    in_maps = _host_inputs(points, anchor_points, cor_score, Wa, ba, Wd, bd)
    res = run_bass_kernel_spmd(nc, in_maps, core_ids=list(range(NCORES)))
    if _timing is not None:
        _timing.append(res.exec_time_ns)
    out = np.empty((N, HIDDEN), np.float32)
    for core in range(NCORES):
        ot = res.results[core]["outT"]          # (2, 128, 512)
        blk = out[core * NC_PTS:(core + 1) * NC_PTS]
        blk[:, 0:128] = ot[0].T
        blk[:, 128:256] = ot[1].T
    return out.reshape(1, N, HIDDEN)


# revision 3
# speedup vs baseline: 1.0047x; 1.0047x over previous
"""Trainium2 Bass kernel for CrossGeometricStructureEmbedding (v3).

Math per point n, anchor k:
  d_idx = |p_n - a_k| / 0.2, a_idx = atan2(|u x v|, u.v) * 180/(15*pi)
  out[n] = max_k(Wd@emb(d_idx)) + max_k(Wa@emb(a_idx)) + bd + ba

Approach (8 cores, 512 points each):
  emb(x)@W.T is compressed through a Fourier-extension basis
  emb(x) ~= B(x) @ C with B_j(x) = sin(2*pi*(f_j*x + phi_j)) at machine
  precision (64 d-rows / 32 a-rows), so each pair needs only 96 sins
  and the projection contraction drops 256 -> 96. No arccos/Chebyshev
  chain is needed: the basis is evaluated directly in dist / angle.

  Per 512-pair chunk (8 points x 64 anchors), engine split is chosen
  from walrus-legal ops only (Pool cannot touch PSUM or do TT-max):
    SP    x broadcast to 96 partitions as one wide stride-0 DMA/proj
    Pool  t = f_j*x + phi_j (per-partition scalars); frac = t - it
    ACT   it = int32(round(t)); basis = Sin(2*pi*frac)
    PE    4 f32r projection matmuls (96 -> 2x256 dims)
    DVE   k-max of both psums (TensorReduce, the only legal max)
  The x rows are relaid out point-major -> pair-major once through
  DRAM (2 write DMAs); per-chunk broadcasts read DRAM directly.
"""
import sys

sys.path.insert(0, "/opt/trn_rl_repo")

import numpy as np
import concourse.bacc as bacc
import concourse.bass as bass
import concourse.tile as tile
from concourse import mybir
from concourse.bass_utils import run_bass_kernel_spmd

F32 = mybir.dt.float32
F32R = mybir.dt.float32r
I32 = mybir.dt.int32
AF = mybir.ActivationFunctionType
OP = mybir.AluOpType

NCORES = 8
N = 4096
NC_PTS = N // NCORES          # 512 points per core
K = 64
HIDDEN = 256
SIGMA_D = 0.2
SIGMA_A = 15.0
FACTOR_A = 180.0 / (SIGMA_A * np.pi)
TWO_PI = float(2.0 * np.pi)

# Fourier-extension basis: emb(x) ~= sin(2pi(f_j x + phi_j)) @ C
J_D, PEXT_D, LO_D, HI_D = 31, 90.0, -0.5, 44.5
J_A, PEXT_A, LO_A, HI_A = 15, 26.0, -0.5, 12.5
M_D, M_A = 2 * J_D + 2, 2 * J_A + 2          # 64, 32
MB = M_D + M_A

_DIV = np.exp(np.arange(0, HIDDEN, 2) * (-np.log(10000.0) / HIDDEN))


def _fourier_rows(J, pext):
    freqs = [0.0]
    phases = [0.25]
    for j in range(1, J + 1):
        freqs += [j / pext, j / pext]
        phases += [0.0, 0.25]
    freqs.append((J + 1) / pext)
    phases.append(0.0)
    return np.array(freqs), np.array(phases)


def _fit(lo, hi, J, pext, grid_n=8000):
    f, p = _fourier_rows(J, pext)
    xg = np.linspace(lo, hi, grid_n)
    B = np.sin(2 * np.pi * (xg[:, None] * f[None, :] + p[None, :]))
    om = xg[:, None] * _DIV
    E = np.stack([np.sin(om), np.cos(om)], -1).reshape(grid_n, HIDDEN)
    C, *_ = np.linalg.lstsq(B, E, rcond=None)
    return C, f, p


_C_D, _F_D, _P_D = _fit(LO_D, HI_D, J_D, PEXT_D)
_C_A, _F_A, _P_A = _fit(LO_A, HI_A, J_A, PEXT_A)

_NC_CACHE = {}


def _build_nc():
    nc = bacc.Bacc("TRN2", target_bir_lowering=False, debug=False,
                   num_devices=NCORES)
    pts = nc.declare_dram_parameter("pts", [128, 12], F32, isOutput=False)
    nab = nc.declare_dram_parameter("nab", [128, 6, K], F32, isOutput=False)
    wlhs = nc.declare_dram_parameter("wlhs", [MB, 512], F32R, isOutput=False)
    sjp = nc.declare_dram_parameter("sjp", [MB, 2], F32, isOutput=False)
    biasd = nc.declare_dram_parameter("biasd", [128, 2], F32, isOutput=False)
    outT = nc.declare_dram_parameter("outT", [2, 128, 512], F32, isOutput=True)

    NCH = 64

    with tile.TileContext(nc) as tc:
        with (
            tc.tile_pool(name="singles", bufs=1) as sg,
            tc.tile_pool(name="geom", bufs=1) as gm,
            tc.tile_pool(name="dram", bufs=1, space="DRAM") as dr,
            tc.tile_pool(name="pj", bufs=2, space="PSUM") as pj,
            tc.tile_pool(name="tbp", bufs=3) as tbp,
            tc.tile_pool(name="ttp", bufs=3) as ttp,
            tc.tile_pool(name="itp", bufs=3) as itp,
            tc.tile_pool(name="rtp", bufs=3) as rtp,
            tc.tile_pool(name="btp", bufs=3) as btp,
        ):
            pts_sb = sg.tile([128, 12], F32, name="pts_sb")
            nab_sb = sg.tile([128, 6, K], F32, name="nab_sb")
            wlhs_sb = sg.tile([MB, 512], F32R, name="wlhs_sb")
            sjp_sb = sg.tile([MB, 2], F32, name="sjp_sb")
            bias_sb = sg.tile([128, 2], F32, name="bias_sb")
            mx_all = sg.tile([128, 4, 512], F32, name="mx_all")
            thd = dr.tile([2, 8, 4096], F32, name="thd")

            nc.sync.dma_start(pts_sb[:], pts[:])
            nc.sync.dma_start(nab_sb[:], nab[:])
            nc.scalar.dma_start(wlhs_sb[:], wlhs[:])
            nc.sync.dma_start(sjp_sb[:], sjp[:])
            nc.sync.dma_start(bias_sb[:], biasd[:])

            # ---------- geometry -------------------------------------------
            # computed in two column slices: g=0 first (feeds rr 0/1 = the
            # first 16 chunks) so the chunk pipeline starts ~6us earlier,
            # then g=1..3 while the early chunks already run
            W = 4 * K  # 256
            u6 = gm.tile([128, 6, W], F32, name="u6")
            engs = [nc.vector, nc.gpsimd]

            m0 = gm.tile([128, 8, W], F32, name="m0")
            ta = gm.tile([128, W], F32, name="ta")
            tb = gm.tile([128, W], F32, name="tb")
            cx = gm.tile([128, W], F32, name="cx")
            cy = gm.tile([128, W], F32, name="cy")
            cz = gm.tile([128, W], F32, name="cz")
            dt_ = gm.tile([128, W], F32, name="dt_")
            xpair = gm.tile([128, 2, W], F32, name="xpair")

            # warm the sqrt act table while the loads are in flight
            nc.scalar.square(ta[0:1, 0:4], pts_sb[0:1, 0:4])
            nc.scalar.activation(tb[0:1, 0:4], ta[0:1, 0:4], AF.Sqrt)

            def u6_fill(gs):
                for c in range(6):
                    for g in gs:
                        engs[(c * 4 + g) % 2].tensor_scalar_add(
                            u6[:, c, g * K:(g + 1) * K],
                            nab_sb[:, c, :],
                            pts_sb[:, g * 3 + (c % 3):g * 3 + (c % 3) + 1],
                        )

            # thd[proj, rr=2g+h, (p%64)*64 + k] = xpair[p, proj, g*64+k]
            def wr(proj, h, g0, g1, eng):
                src = xpair[64 * h:64 * h + 64, proj,
                            g0 * K:g1 * K].rearrange("p (g k) -> p g k", k=K)
                dst = thd[proj].rearrange(
                    "(g h) (p k) -> h p g k", h=2, k=K)[h][:, g0:g1, :]
                eng.dma_start(out=dst, in_=src)

            def geo_chain(cs, first):
                # cross products (Pool mults, DVE subs)
                nc.gpsimd.tensor_tensor(out=m0[:, 0, cs], in0=u6[:, 1, cs], in1=u6[:, 5, cs], op=OP.mult)
                nc.gpsimd.tensor_tensor(out=m0[:, 1, cs], in0=u6[:, 2, cs], in1=u6[:, 4, cs], op=OP.mult)
                nc.gpsimd.tensor_tensor(out=m0[:, 2, cs], in0=u6[:, 2, cs], in1=u6[:, 3, cs], op=OP.mult)
                nc.gpsimd.tensor_tensor(out=m0[:, 3, cs], in0=u6[:, 0, cs], in1=u6[:, 5, cs], op=OP.mult)
                nc.gpsimd.tensor_tensor(out=m0[:, 4, cs], in0=u6[:, 0, cs], in1=u6[:, 4, cs], op=OP.mult)
                nc.gpsimd.tensor_tensor(out=m0[:, 5, cs], in0=u6[:, 1, cs], in1=u6[:, 3, cs], op=OP.mult)
                nc.vector.tensor_tensor(out=cx[:, cs], in0=m0[:, 0, cs], in1=m0[:, 1, cs], op=OP.subtract)
                nc.vector.tensor_tensor(out=cy[:, cs], in0=m0[:, 2, cs], in1=m0[:, 3, cs], op=OP.subtract)
                nc.vector.tensor_tensor(out=cz[:, cs], in0=m0[:, 4, cs], in1=m0[:, 5, cs], op=OP.subtract)
                # dist chain in parallel (ACT squares + DVE adds)
                nc.scalar.square(ta[:, cs], u6[:, 0, cs])
                nc.scalar.square(tb[:, cs], u6[:, 1, cs])
                nc.vector.tensor_tensor(out=ta[:, cs], in0=ta[:, cs], in1=tb[:, cs], op=OP.add)
                nc.scalar.square(tb[:, cs], u6[:, 2, cs])
                nc.vector.tensor_tensor(out=ta[:, cs], in0=ta[:, cs], in1=tb[:, cs], op=OP.add)
                nc.scalar.activation(xpair[:, 0, cs], ta[:, cs], AF.Sqrt)
                # dot products and |cross|^2 (Pool)
                nc.gpsimd.tensor_tensor(out=m0[:, 6, cs], in0=u6[:, 0, cs], in1=u6[:, 3, cs], op=OP.mult)
                nc.gpsimd.tensor_tensor(out=m0[:, 7, cs], in0=u6[:, 1, cs], in1=u6[:, 4, cs], op=OP.mult)
                nc.gpsimd.tensor_tensor(out=tb[:, cs], in0=u6[:, 2, cs], in1=u6[:, 5, cs], op=OP.mult)
                nc.gpsimd.tensor_tensor(out=dt_[:, cs], in0=m0[:, 6, cs], in1=m0[:, 7, cs], op=OP.add)
                nc.gpsimd.tensor_tensor(out=dt_[:, cs], in0=dt_[:, cs], in1=tb[:, cs], op=OP.add)
                nc.gpsimd.tensor_tensor(out=cx[:, cs], in0=cx[:, cs], in1=cx[:, cs], op=OP.mult)
                nc.gpsimd.tensor_tensor(out=cy[:, cs], in0=cy[:, cs], in1=cy[:, cs], op=OP.mult)
                nc.gpsimd.tensor_tensor(out=cx[:, cs], in0=cx[:, cs], in1=cy[:, cs], op=OP.add)
                nc.gpsimd.tensor_tensor(out=cz[:, cs], in0=cz[:, cs], in1=cz[:, cs], op=OP.mult)
                nc.gpsimd.tensor_tensor(out=cx[:, cs], in0=cx[:, cs], in1=cz[:, cs], op=OP.add)
                nc.scalar.activation(cy[:, cs], cx[:, cs], AF.Sqrt)    # |cross|
                # angle = arctan(|cross|/dot) + pi*(dot<0)
                nc.vector.reciprocal(cz[:, cs], dt_[:, cs])
                nc.vector.tensor_tensor(out=cy[:, cs], in0=cy[:, cs], in1=cz[:, cs], op=OP.mult)
                nc.scalar.activation(cy[:, cs], cy[:, cs], AF.Arctan)
                if first:
                    # pin the Sin table load here, off the chunk critical path
                    nc.scalar.activation(tb[0:1, 0:1], bias_sb[0:1, 0:1],
                                         AF.Sin)
                nc.vector.tensor_scalar(out=cz[:, cs], in0=dt_[:, cs],
                                        scalar1=0.0, scalar2=None, op0=OP.is_lt)
                nc.vector.scalar_tensor_tensor(out=xpair[:, 1, cs],
                                               in0=cz[:, cs],
                                               scalar=float(np.pi),
                                               in1=cy[:, cs],
                                               op0=OP.mult, op1=OP.add)

            u6_fill([0])
            geo_chain(slice(0, K), True)
            wr(0, 0, 0, 1, nc.sync)
            wr(0, 1, 0, 1, nc.scalar)
            wr(1, 0, 0, 1, nc.sync)
            wr(1, 1, 0, 1, nc.scalar)
            u6_fill([1, 2, 3])
            geo_chain(slice(K, W), False)
            wr(0, 0, 1, 4, nc.sync)
            wr(0, 1, 1, 4, nc.scalar)
            wr(1, 0, 1, 4, nc.sync)
            wr(1, 1, 1, 4, nc.scalar)

            # ---------- pipelined chunk loop: 64 chunks of 512 pairs --------
            tb6 = {}
            tt = {}
            it = {}
            rt = {}
            bt = {}
            psA = {}
            psB = {}

            def s_bc(c):
                # broadcast this chunk's x rows to 96 partitions (stride-0)
                rr, cc = divmod(c, 8)
                t_ = tbp.tile([MB, 512], F32, name="tb6")
                for proj, (p0, cnt) in enumerate(((0, M_D), (M_D, M_A))):
                    srow = thd[proj, rr, :]
                    ap = bass.AP(tensor=srow.tensor,
                                 offset=srow.offset + cc * 512,
                                 ap=[[0, cnt], [1, 512]])
                    nc.sync.dma_start(out=t_[p0:p0 + cnt, :], in_=ap)
                tb6[c] = t_

            def s_aff(c):
                # t = f_j * x + phi_j  (per-partition scalars, on Pool)
                tt_ = ttp.tile([MB, 512], F32, name="tt")
                nc.gpsimd.tensor_scalar(out=tt_[:], in0=tb6[c][:],
                                        scalar1=sjp_sb[:, 0:1],
                                        scalar2=sjp_sb[:, 1:2],
                                        op0=OP.mult, op1=OP.add)
                tt[c] = tt_
                tb6.pop(c)

            def s_cast(c):
                it_ = itp.tile([MB, 512], I32, name="it")
                nc.scalar.copy(it_[:], tt[c][:])
                it[c] = it_

            def s_sub(c):
                rt_ = rtp.tile([MB, 512], F32, name="rt")
                nc.gpsimd.tensor_tensor(out=rt_[:], in0=tt[c][:],
                                        in1=it[c][:], op=OP.subtract)
                rt[c] = rt_
                tt.pop(c)
                it.pop(c)

            def s_sin(c):
                bt_ = btp.tile([MB, 512], F32R, name="bt")
                nc.scalar.activation(bt_[:], rt[c][:], AF.Sin, scale=TWO_PI)
                bt[c] = bt_
                rt.pop(c)

            def s_proj(c):
                # all 4 projection matmuls into one 4-bank psum tile so the
                # k-max is a single TensorReduce
                ps_ = pj.tile([128, 2048], F32, name="ps")
                b = bt[c]
                nc.tensor.matmul(ps_[:, 0:512], wlhs_sb[0:M_D, 0:128],
                                 b[0:M_D, :], start=True, stop=True)
                nc.tensor.matmul(ps_[:, 512:1024], wlhs_sb[0:M_D, 128:256],
                                 b[0:M_D, :], start=True, stop=True)
                nc.tensor.matmul(ps_[:, 1024:1536], wlhs_sb[M_D:MB, 256:384],
                                 b[M_D:MB, :], start=True, stop=True)
                nc.tensor.matmul(ps_[:, 1536:2048], wlhs_sb[M_D:MB, 384:512],
                                 b[M_D:MB, :], start=True, stop=True)
                psA[c] = ps_

            def s_red(c):
                v = psA[c].rearrange("p (t n k) -> p t n k", t=4, k=K)
                nc.vector.tensor_reduce(
                    mx_all[:, :, c * 8:(c + 1) * 8], v[:],
                    axis=mybir.AxisListType.X, op=OP.max)
                psA.pop(c)
                bt.pop(c)

            o0 = gm.tile([128, 512], F32, name="o0")
            o1 = gm.tile([128, 512], F32, name="o1")

            def s_fin(g8):
                cs = slice(g8 * 32, (g8 + 1) * 32)
                nc.gpsimd.tensor_tensor(out=o0[:, cs], in0=mx_all[:, 0, cs],
                                        in1=mx_all[:, 2, cs], op=OP.add)
                nc.gpsimd.tensor_scalar_add(o0[:, cs], o0[:, cs],
                                            bias_sb[:, 0:1])
                nc.gpsimd.tensor_tensor(out=o1[:, cs], in0=mx_all[:, 1, cs],
                                        in1=mx_all[:, 3, cs], op=OP.add)
                nc.gpsimd.tensor_scalar_add(o1[:, cs], o1[:, cs],
                                            bias_sb[:, 1:2])
                nc.scalar.dma_start(out=outT[0, :, cs], in_=o0[:, cs])
                nc.scalar.dma_start(out=outT[1, :, cs], in_=o1[:, cs])

            for s in range(NCH + 5):
                if 5 <= s:
                    s_red(s - 5)
                    if (s - 5) % 4 == 3:
                        s_fin((s - 5) // 4)
                if 1 <= s <= NCH:
                    s_aff(s - 1)
                if 2 <= s <= NCH + 1:
                    s_cast(s - 2)
                    s_sub(s - 2)
                if 3 <= s <= NCH + 2:
                    s_sin(s - 3)
                if s < NCH:
                    s_bc(s)
                if 4 <= s <= NCH + 3:
                    s_proj(s - 4)

    nc.compile()
    return nc


def _host_inputs(points, anchor_points, cor_score, Wa, ba, Wd, bd):
    p = np.ascontiguousarray(points[0], dtype=np.float32)         # (4096, 3)
    a = np.ascontiguousarray(anchor_points[0], dtype=np.float32)  # (64, 3)

    nab = np.empty((128, 6, K), np.float32)
    nab[:, 0:3, :] = -a.T[None, :, :]
    nab[:, 3:6, :] = -np.roll(a, -1, axis=0).T[None, :, :]

    G_d = (_C_D @ np.asarray(Wd, np.float64).T).astype(np.float32)  # (64, 256)
    G_a = (_C_A @ np.asarray(Wa, np.float64).T).astype(np.float32)  # (32, 256)
    wlhs = np.zeros((MB, 512), np.float32)
    wlhs[0:M_D, 0:128] = G_d[:, 0:128]
    wlhs[0:M_D, 128:256] = G_d[:, 128:256]
    wlhs[M_D:MB, 256:384] = G_a[:, 0:128]
    wlhs[M_D:MB, 384:512] = G_a[:, 128:256]

    sjp = np.zeros((MB, 2), np.float32)
    sjp[0:M_D, 0] = _F_D / SIGMA_D       # basis in dist units
    sjp[M_D:MB, 0] = _F_A * FACTOR_A     # basis in raw-angle units
    sjp[0:M_D, 1] = _P_D
    sjp[M_D:MB, 1] = _P_A

    bsum = (np.asarray(bd) + np.asarray(ba)).astype(np.float32)
    biasd = np.stack([bsum[0:128], bsum[128:256]], axis=1).copy()  # (128, 2)

    in_maps = []
    for core in range(NCORES):
        pc = p[core * NC_PTS:(core + 1) * NC_PTS]   # (512, 3)
        ptsv = pc.reshape(4, 128, 3).transpose(1, 0, 2).reshape(128, 12)
        in_maps.append({
            "pts": np.ascontiguousarray(ptsv),
            "nab": nab,
            "wlhs": wlhs,
            "sjp": sjp,
            "biasd": biasd,
        })
    return in_maps


def kernel(points, anchor_points, cor_score, Wa, ba, Wd, bd, _timing=None):
    if "nc" not in _NC_CACHE:
        _NC_CACHE["nc"] = _build_nc()
    nc = _NC_CACHE["nc"]
    in_maps = _host_inputs(points, anchor_points, cor_score, Wa, ba, Wd, bd)
    res = run_bass_kernel_spmd(nc, in_maps, core_ids=list(range(NCORES)))
    if _timing is not None:
        _timing.append(res.exec_time_ns)
    out = np.empty((N, HIDDEN), np.float32)
    for core in range(NCORES):
        ot = res.results[core]["outT"]          # (2, 128, 512)
        blk = out[core * NC_PTS:(core + 1) * NC_PTS]
        blk[:, 0:128] = ot[0].T
        blk[:, 128:256] = ot[1].T
    return out.reshape(1, N, HIDDEN)
